# revision 1
# baseline (speedup 1.0000x reference)
"""Trainium2 Bass kernel for nn_BinaryLayer (logic-gate network).

Computes: out[b, o] = OR_t AND_a x_in[b, weights[o, t, a]]
where x_in = [const_true | (x != 0) | ~(x != 0)]  (width 1 + 2*784 = 1569),
plus an or-mask: an (o, t) gate whose 16 indices are all 0 is forced False.

Strategy (8 NeuronCores, tensor-parallel over OUT: 128 outs per core):
  1. On device, bit-pack x across batch: T16[g, f] = sum_j x[16g+j, f] * 2^j
     via PE matmuls (exact in fp32).  B=1024 -> 64 uint16 groups.
  2. Build a 1570-row boolean table in SBUF, laid out for ap_gather:
     partition 16c+l holds, for each table row, batches [64l, 64l+64) as
     4 uint16 (built with PE permutation matmuls; rows: 0=const-true,
     1..784 = x, 785..1568 = ~x, 1569 = const-false for masked gates).
  3. GPSIMD ap_gather: each Q7 core c gathers its 8192 gate-term rows
     (16 outs x 32 or-terms x 16 and-terms) into its 16 partitions.
  4. DVE bitwise AND tree over and-terms, OR tree over or-terms.
  5. DMA 16KB of packed result bits out; host unpacks to bool.
"""

import numpy as np

B, F = 1024, 784
OUT, OR_T, AND_T = 1024, 32, 16
NR = 1570  # table rows
N_CORES = 8

_cache = {}


def _build(reps=1):
    import concourse.bass as bass
    import concourse.mybir as mybir
    import concourse.tile as tile
    from concourse.bacc import Bacc

    f32 = mybir.dt.float32
    u16 = mybir.dt.uint16
    i16 = mybir.dt.int16
    u32 = mybir.dt.uint32
    Alu = mybir.AluOpType

    nc = Bacc("TRN2", target_bir_lowering=False, debug=False, num_devices=N_CORES)
    x_t = nc.dram_tensor("x", [B, F], f32, kind="ExternalInput")
    packw_t = nc.dram_tensor("packw", [128, 8, 64], f32, kind="ExternalInput")
    pperm_t = nc.dram_tensor("pperm", [64, 512], f32, kind="ExternalInput")
    idxs_t = nc.dram_tensor("idxs", [128, 512], i16, kind="ExternalInput")
    out_t = nc.dram_tensor("out", [128, 64], u16, kind="ExternalOutput")

    with tile.TileContext(nc) as tc:
        with (
            tc.tile_pool(name="main", bufs=1) as pool,
            tc.tile_pool(name="gp", bufs=1) as gpool,
            tc.tile_pool(name="psum", bufs=2, space="PSUM") as pp,
        ):
            x_sb = pool.tile([128, 8, F], f32)
            packw_sb = pool.tile([128, 8, 64], f32)
            pperm_sb = pool.tile([64, 512], f32)
            idx_sb = pool.tile([128, 512], i16)
            x_r = x_t.ap().rearrange("(t p) f -> p t f", p=128)
            for kt in range(8):
                nc.sync.dma_start(x_sb[:, kt, :], x_r[:, kt, :])
            nc.sync.dma_start(packw_sb[:], packw_t.ap())
            nc.sync.dma_start(pperm_sb[:], pperm_t.ap())
            nc.sync.dma_start(idx_sb[:], idxs_t.ap())

            for _rep in range(reps):
                # --- step 1: bit-pack x into T16 [64 groups, 784 features] ---
                sbufT = pool.tile([64, NR + 2], f32, tag="sbufT")
                psumA0 = pp.tile([64, 392], f32, tag="psumA0")
                psumA1 = pp.tile([64, 392], f32, tag="psumA1")
                for kt in range(8):
                    for h, ps in ((0, psumA0), (1, psumA1)):
                        nc.tensor.matmul(
                            out=ps[:],
                            lhsT=packw_sb[:, kt, :],
                            rhs=x_sb[:, kt, 392 * h : 392 * (h + 1)],
                            start=(kt == 0),
                            stop=(kt == 7),
                        )
                # --- step 2: full table row values in fp32 [64, 1570] ---
                nc.vector.memset(sbufT[:, 0:1], 65535.0)
                nc.vector.tensor_copy(out=sbufT[:, 1:393], in_=psumA0[:])
                nc.vector.tensor_copy(out=sbufT[:, 393:785], in_=psumA1[:])
                nc.vector.tensor_scalar(
                    out=sbufT[:, 785:1177], in0=psumA0[:],
                    scalar1=-1.0, scalar2=65535.0, op0=Alu.mult, op1=Alu.add,
                )
                nc.vector.tensor_scalar(
                    out=sbufT[:, 1177:1569], in0=psumA1[:],
                    scalar1=-1.0, scalar2=65535.0, op0=Alu.mult, op1=Alu.add,
                )
                nc.vector.memset(sbufT[:, 1569:1570], 0.0)

                # --- replicate/regroup into gather layout via PE permutation ---
                # table[16c+l, r, j] = T16[4l+j, r] (as uint16)
                table = pool.tile([128, NR, 4], u16, tag="table")
                for j in range(4):
                    for lo, hi in ((0, 400), (400, 800), (800, 1200), (1200, 1570)):
                        psumB = pp.tile([128, 400], f32, tag="psumB")
                        nc.tensor.matmul(
                            out=psumB[:, : hi - lo],
                            lhsT=pperm_sb[:, 128 * j : 128 * (j + 1)],
                            rhs=sbufT[:, lo:hi],
                            start=True,
                            stop=True,
                        )
                        nc.vector.tensor_copy(
                            out=table[:, lo:hi, j], in_=psumB[:, : hi - lo]
                        )

                # --- steps 3-4: gather + AND tree + OR tree, two halves ---
                outres = pool.tile([128, 16, 4], u16, tag="outres")
                for half in range(1):
                    gath = gpool.tile([128, 8192, 2], u32, tag="gath")
                    nc.gpsimd.ap_gather(
                        gath[:],
                        table[:].bitcast(u32),
                        idx_sb[:],
                        channels=128,
                        num_elems=NR,
                        d=2,
                        num_idxs=8192,
                    )
                    gv = gath[:].bitcast(u16).rearrange("p (g a) w -> p g a w", a=16)
                    and1 = gpool.tile([128, 512, 8, 4], u16, tag="and1")
                    and2 = gpool.tile([128, 512, 4, 4], u16, tag="and2")
                    and3 = gpool.tile([128, 512, 2, 4], u16, tag="and3")
                    and4 = gpool.tile([128, 512, 1, 4], u16, tag="and4")
                    nc.vector.tensor_tensor(
                        out=and1[:], in0=gv[:, :, 0::2, :], in1=gv[:, :, 1::2, :],
                        op=Alu.bitwise_and,
                    )
                    nc.vector.tensor_tensor(
                        out=and2[:], in0=and1[:, :, 0::2, :], in1=and1[:, :, 1::2, :],
                        op=Alu.bitwise_and,
                    )
                    nc.vector.tensor_tensor(
                        out=and3[:], in0=and2[:, :, 0::2, :], in1=and2[:, :, 1::2, :],
                        op=Alu.bitwise_and,
                    )
                    nc.vector.tensor_tensor(
                        out=and4[:], in0=and3[:, :, 0::2, :], in1=and3[:, :, 1::2, :],
                        op=Alu.bitwise_and,
                    )
                    # OR over t: and4 viewed [128, 8 outs, 32 t, 4]
                    ov = and4[:].rearrange("p (o t) a w -> p o (t a) w", t=32)
                    or1 = gpool.tile([128, 16, 16, 4], u16, tag="or1")
                    or2 = gpool.tile([128, 16, 8, 4], u16, tag="or2")
                    or3 = gpool.tile([128, 16, 4, 4], u16, tag="or3")
                    or4 = gpool.tile([128, 16, 2, 4], u16, tag="or4")
                    nc.vector.tensor_tensor(
                        out=or1[:], in0=ov[:, :, 0::2, :], in1=ov[:, :, 1::2, :],
                        op=Alu.bitwise_or,
                    )
                    nc.vector.tensor_tensor(
                        out=or2[:], in0=or1[:, :, 0::2, :], in1=or1[:, :, 1::2, :],
                        op=Alu.bitwise_or,
                    )
                    nc.vector.tensor_tensor(
                        out=or3[:], in0=or2[:, :, 0::2, :], in1=or2[:, :, 1::2, :],
                        op=Alu.bitwise_or,
                    )
                    nc.vector.tensor_tensor(
                        out=or4[:], in0=or3[:, :, 0::2, :], in1=or3[:, :, 1::2, :],
                        op=Alu.bitwise_or,
                    )
                    nc.vector.tensor_tensor(
                        out=outres[:].unsqueeze(2),
                        in0=or4[:, :, 0:1, :],
                        in1=or4[:, :, 1:2, :],
                        op=Alu.bitwise_or,
                    )
                nc.sync.dma_start(
                    out_t.ap(), outres[:].rearrange("p o w -> p (o w)")
                )
    nc.compile()
    return nc


def _wrap16(flat):
    """Flat per-core idx list -> the Q7 16-partition wrapped layout."""
    k = flat.shape[0]
    return flat.reshape(k // 32, 2, 16).transpose(2, 0, 1).reshape(16, k // 16)


def _host_inputs(x, weights):
    x = np.ascontiguousarray(np.asarray(x, dtype=np.float32))
    w = np.asarray(weights).astype(np.int64)  # [1024, 32, 16]

    # pack weights: packW[b, g] = 2^(b%16) if b//16 == g
    b_idx = np.arange(B)
    packw = np.zeros((B, 64), np.float32)
    packw[b_idx, b_idx // 16] = (2.0 ** (b_idx % 16)).astype(np.float32)
    packw_in = packw.reshape(8, 128, 64).transpose(1, 0, 2).copy()  # [p, kt, g]

    # permutation matrices: P_j[k, 128j + m] = [k == 4*(m%16) + j]
    pperm = np.zeros((64, 512), np.float32)
    for j in range(4):
        m = np.arange(128)
        pperm[4 * (m % 16) + j, 128 * j + m] = 1.0

    # gate index lists, per chip-core
    allzero = (w == 0).all(-1)  # [1024, 32]
    wr = np.where(allzero[:, :, None], NR - 1, w).astype(np.int16)  # [1024, 32, 16]
    idx_maps = []
    for cc in range(N_CORES):
        rows = np.zeros((128, 512), np.int16)
        for c in range(8):
            o_base = 128 * cc + 16 * c
            flat = wr[o_base : o_base + 16].reshape(-1)  # [16*32*16 = 8192]
            rows[16 * c : 16 * (c + 1)] = _wrap16(flat)
        idx_maps.append(rows)
    return x, packw_in, pperm, idx_maps


def _assemble(results):
    out = np.zeros((B, OUT), dtype=bool)
    for cc in range(N_CORES):
        o16 = np.ascontiguousarray(results[cc]["out"]).view(np.uint16)
        o16 = o16.reshape(128, 16, 4)  # [p=16c+l, o_local, j]
        bits = np.unpackbits(
            o16.astype("<u2").view(np.uint8).reshape(128, 16, 4, 2),
            axis=-1,
            bitorder="little",
        ).reshape(128, 16, 4, 16)  # [p, ol, j, bit]
        a = bits.reshape(8, 16, 16, 4, 16)  # [c, l, ol, j, bit]
        # batch = 64l + 16j + bit ; out col = 128cc + 16c + ol
        blk = a.transpose(1, 3, 4, 0, 2).reshape(B, 128)
        out[:, 128 * cc : 128 * (cc + 1)] = blk.astype(bool)
    return out


def kernel(x, weights):
    from concourse.bass_utils import run_bass_kernel_spmd

    if "nc" not in _cache:
        _cache["nc"] = _build(reps=1)
    nc = _cache["nc"]

    xf, packw_in, pperm, idx_maps = _host_inputs(x, weights)
    in_maps = [
        {"x": xf, "packw": packw_in, "pperm": pperm, "idxs": idx_maps[cc]}
        for cc in range(N_CORES)
    ]
    try:
        res = run_bass_kernel_spmd(nc, in_maps, core_ids=list(range(N_CORES)))
    except Exception:
        # transient device/tunnel errors: retry once on a fresh attempt
        res = run_bass_kernel_spmd(nc, in_maps, core_ids=list(range(N_CORES)))
    return _assemble(res.results)

